# revision 47
# baseline (speedup 1.0000x reference)
"""Trainium2 Bass kernel for Swin-style window attention (MegatronWindowAttention).

Per window w (49 tokens, dim 256, 8 heads x 32):
  qkv = x @ qkv_w.T + qkv_b ; attn = softmax(q*scale @ k.T + bias + mask[w%64])
  out = (attn @ v) @ proj_w.T + proj_b

Sharding: data-parallel over B_=4096 windows across 8 cores (512 windows each).

v2 device dataflow per core (4 blocks of 128 windows; window PAIRS of 98 tokens):
  - x.T arrives fp8 (e4m3) kb-interleaved [128, 2, T]; host pre-transposed
  - Q.T/K.T staged per 8-pair group via fp8 DoubleRow W-stationary matmuls
    (both 128-ic halves in one MM); PSUM evac: q on DVE (scale+bias), k on ACT
  - Per pair: log-domain rel-pos bias (+mask, -30 cross-window kill) PRELOADED
    into the S.T PSUM banks via identity matmuls; S.T matmuls accumulate on
    top (4 row-band-concurrent per bank); ONE ACT exp -> pm (attn weights)
  - V per pair via ONE fp8 DoubleRow MM (slotted 33-wide per head; ones col
    comes free from the bias add); evac fused scale+bias on DVE
  - AV per head: lhsT = pm 128-col slot (FWL), rhs = V_aug -> O + rowsum
  - normalize via DVE reciprocal+mul; PE transposes (2/pair) -> O.T packed
    per-group; proj is W-stationary per half-group (4 MMs of 392 cols)
    producing OUT.T; ACT evac (+proj bias per-partition); bf16 DMA out.
  - host reassembles out.T -> [B, N, 256] f32
"""

import os
import numpy as np
import ml_dtypes
from contextlib import ExitStack

import concourse.bass as bass
import concourse.tile as tile
import concourse.mybir as mybir
from concourse import bacc
from concourse import bass_utils

FP8_EN = os.environ.get("FP8", "1") == "1"

WH = WW = 7
NTOK = 49
DIM = 256
NH = 8
HD = 32
SCALE = HD ** -0.5
NCORES = 8
B_FULL = 4096
NW = 64
B_CORE = B_FULL // NCORES          # 512 windows per core
T_CORE = B_CORE * NTOK             # 25088 tokens
NBLOCK = 4
W_BLK = 128                        # windows per block
T_BLK = W_BLK * NTOK               # 6272 tokens per block
T_PAD = T_BLK + 32                 # tail pad for 128-col stationary reads
NPAIR = W_BLK // 2                 # 64 pairs per block
GPAIR = 8                          # pairs per QK staging group
NGRP = NPAIR // GPAIR              # 8 groups per block
GW = GPAIR * 98                    # 784 tokens per group
GWH = GW + 32                      # halo for 128-col K stationary reads
GCH = GWH // 2                     # 408: QK psum chunk width
WSC = 32.0                         # fp8 weight pre-scale (folded out in evac)
WSCI = 1.0 / WSC

F32 = mybir.dt.float32
BF16 = mybir.dt.bfloat16
FP8 = mybir.dt.float8e4
AF = mybir.ActivationFunctionType
ALU = mybir.AluOpType
DR = mybir.MatmulPerfMode.DoubleRow


def _rel_pos_index():
    coords = np.stack(np.meshgrid(np.arange(WH), np.arange(WW), indexing='ij'))
    flat = coords.reshape(2, -1)
    rel = flat[:, :, None] - flat[:, None, :]
    rel = rel.transpose(1, 2, 0).copy()
    rel[:, :, 0] += WH - 1
    rel[:, :, 1] += WW - 1
    rel[:, :, 0] *= 2 * WW - 1
    return rel.sum(-1).reshape(-1)


def build_kernel(ctx: ExitStack, tc: tile.TileContext, ins: dict, outT: bass.AP,
                 mask_zero: bool, fp8: bool):
    nc = tc.nc
    XDT = FP8 if fp8 else BF16
    xin = ins["xin"]        # [128, 2, T_CORE] fp8/bf16 (x.T, kb-interleaved)
    xrn = ins.get("xrn")    # [128, 2, T_CORE] fp8 residual x.T (fp8 path only)
    qkw = ins["qkw"]        # [128, 2, 4, 128] fp8/bf16 (ic, kb, ob(q0 q1 k0 k1), oc)
    wv = ins["wv"]          # [128, 2, 272] fp8/bf16 (ic, kb, slotted oc)
    wvr = ins.get("wvr")    # [128, 2, 272] fp8 residual V weights (fp8 path only)
    pwt4 = ins["pwt4"]      # [128, 2, 2, 128] bf16 proj W.T chunks (ic-chunk, oc-chunk)
    ebl = ins["ebl"]        # log-domain bias [128, 2, 392] (fast) or [128, 32, 2, 392]
    idb = ins["identb"]     # [128, 128] bf16
    qkvb = ins["qkvb"]      # [128, 4] f32 per-partition bias for q0 q1 k0 k1
    vbb = ins["vbb"]        # [128, 264] f32 broadcast v-bias (+1.0 ones slots)
    pbT = ins["pbT"]        # [128, 2] f32 proj bias transposed chunks

    const = ctx.enter_context(tc.tile_pool(name="const", bufs=1))
    qkw_sb = const.tile([128, 2 * 4 * 128], XDT, tag="qkw")
    nc.sync.dma_start(qkw_sb[:], qkw.rearrange("p a b c -> p (a b c)"))
    qkw_v = qkw_sb[:].rearrange("p (a b c) -> p a b c", a=2, b=4)
    wv_sb = const.tile([128, 2 * 272], XDT, tag="wv")
    nc.sync.dma_start(wv_sb[:], wv.rearrange("p a c -> p (a c)"))
    wv_v = wv_sb[:].rearrange("p (a c) -> p a c", a=2)
    if fp8:
        wvr_sb = const.tile([128, 2 * 272], FP8, tag="wvr")
        nc.sync.dma_start(wvr_sb[:], wvr.rearrange("p a c -> p (a c)"))
        wvr_v = wvr_sb[:].rearrange("p (a c) -> p a c", a=2)
    pwt_sb = const.tile([128, 4 * 128], BF16, tag="pwt4")
    nc.sync.dma_start(pwt_sb[:], pwt4.rearrange("p a b c -> p (a b c)"))
    idb_sb = const.tile([128, 128], BF16, tag="identb")
    nc.sync.dma_start(idb_sb[:], idb[:])
    qkvb_sb = const.tile([128, 4], F32, tag="qkvb")
    nc.sync.dma_start(qkvb_sb[:], qkvb[:])
    vbb_sb = const.tile([128, 264], F32, tag="vbb")
    nc.sync.dma_start(vbb_sb[:], vbb[:])
    pbT_sb = const.tile([128, 2], F32, tag="pbT")
    nc.sync.dma_start(pbT_sb[:], pbT[:])
    if mask_zero:
        ebl_sb = const.tile([128, 784], BF16, tag="ebm")
        nc.sync.dma_start(ebl_sb[:], ebl.rearrange("p a c -> p (a c)"))
    else:
        ebl_sb = const.tile([128, 32 * 784], BF16, tag="ebm32")
        nc.sync.dma_start(ebl_sb[:], ebl.rearrange("p m c -> p (m c)"))

    xt_pool = ctx.enter_context(tc.tile_pool(name="xt", bufs=2))
    qk_pool = ctx.enter_context(tc.tile_pool(name="qk", bufs=2))
    pm_pool = ctx.enter_context(tc.tile_pool(name="pm", bufs=2))
    pex_pool = ctx.enter_context(tc.tile_pool(name="pex", bufs=2))
    v_pool = ctx.enter_context(tc.tile_pool(name="vsb", bufs=4))
    r_pool = ctx.enter_context(tc.tile_pool(name="rsb", bufs=4))
    on_pool = ctx.enter_context(tc.tile_pool(name="onorm", bufs=4))
    og_pool = ctx.enter_context(tc.tile_pool(name="otsbG", bufs=2))
    osb_pool = ctx.enter_context(tc.tile_pool(name="osbG", bufs=2))

    # 8 PSUM banks: sps 1x4 (S.T + preloaded bias; concurrent row-band MMs
    # must hit DISTINCT banks -> bank r holds heads (r, r+4)), qkp 2x1
    # (staging + proj), work 2x1 (vw -> avw -> otw rotation)
    ps_s = ctx.enter_context(tc.tile_pool(name="pss", bufs=1, space="PSUM"))
    ps_qk = ctx.enter_context(tc.tile_pool(name="psqk", bufs=2, space="PSUM"))
    ps_w = ctx.enter_context(tc.tile_pool(name="psw", bufs=2, space="PSUM"))

    hist = [None]   # duo-level software pipeline (back half one duo late)

    def back_duo(st):
        pm2 = st["pm2"]
        for p2, sub in enumerate(st["pairs"]):
            vsb = sub["vsb"]
            avw = ps_w.tile([128, 512], F32, tag="work", name="av_" + sub["tag"])
            for h in range(NH):
                s_h = 2 * (h % 4) + h // 4   # pm slot holding head h
                nc.tensor.matmul(
                    avw[:, 33 * h:33 * h + 33],
                    pm2[0:98, 832 * p2 + 98 * s_h:832 * p2 + 98 * s_h + 128],
                    vsb[0:98, 33 * h:33 * h + 33], start=True, stop=True)
            av_v = avw[:, 0:264].rearrange("p (h c) -> p h c", h=8)
            recip = r_pool.tile([128, 8], F32, tag="recip")
            nc.vector.reciprocal(recip[0:98, :], av_v[0:98, :, 32])
            onorm = on_pool.tile([128, 256], BF16, tag="onorm")
            onorm_v = onorm[:].rearrange("p (h c) -> p h c", h=8)
            recip_b = recip[0:98, :].unsqueeze(2).broadcast_to([98, 8, 32])
            nc.vector.tensor_mul(onorm_v[0:98, :, :], av_v[0:98, :, 0:32], recip_b)
            sub["onorm"] = onorm
        # O.T for both pairs into one bank: col = 196*p2 + 98*hb
        otw = ps_w.tile([128, 512], F32, tag="work", name="ot_" + st["tag"])
        otp = otw[:].bitcast(BF16)
        for p2, sub in enumerate(st["pairs"]):
            for hb in range(2):
                nc.tensor.transpose(
                    otp[:, 196 * p2 + 98 * hb:196 * p2 + 98 * hb + 98],
                    sub["onorm"][0:98, 128 * hb:128 * (hb + 1)],
                    idb_sb[0:98, 0:98])
        # evacuate to the group O.T buffer (ch-half major), DVE 2x bf16
        in_v = otp[:, 0:392].rearrange("p (b a c) -> p a b c", b=2, a=2)
        og_v = st["otsbG"][:].rearrange("p (a c) -> p a c", a=2)
        d = st["d"]
        out_v = og_v[:, :, 196 * d:196 * d + 196].rearrange(
            "p a (b c) -> p a b c", b=2)
        nc.vector.tensor_copy(out_v, in_v)

    def emit_proj(st):
        # W-stationary proj for one half-group (392 tokens) -> OUT.T, evac, DMA
        hg = st["d"] // 2
        og_v = st["otsbG"][:].rearrange("p (a c) -> p a c", a=2)
        osb = osb_pool.tile([128, 2 * 392], BF16, tag="osbG")
        osb_v = osb[:].rearrange("p (a c) -> p a c", a=2)
        for o in range(2):
            pj = ps_qk.tile([128, 512], F32, tag="qkp",
                            name=f"pj{o}_" + st["tag"])
            for i in range(2):
                nc.tensor.matmul(
                    pj[:, 0:392], pwt_sb[:, 128 * (2 * i + o):128 * (2 * i + o + 1)],
                    og_v[:, i, 392 * hg:392 * (hg + 1)],
                    start=(i == 0), stop=(i == 1))
            nc.scalar.activation(osb_v[:, o, :], pj[:, 0:392], AF.Identity,
                                 bias=pbT_sb[:, o:o + 1])
        gt0 = st["gt0"] + 392 * hg
        for o in range(2):
            nc.sync.dma_start(outT[:, o, gt0:gt0 + 392], osb_v[:, o, :])

    for blk in range(NBLOCK):
        t0 = blk * T_BLK
        Xt = xt_pool.tile([128, 2 * T_PAD], XDT, tag="xt", name=f"xt_{blk}")
        Xt_v = Xt[:].rearrange("p (a c) -> p a c", a=2)
        nc.gpsimd.memset(Xt_v[:, :, T_BLK:T_PAD], 0.0)
        for kb in range(2):
            nc.sync.dma_start(Xt_v[:, kb, 0:T_BLK], xin[:, kb, t0:t0 + T_BLK])
        if fp8:
            Xtr = xt_pool.tile([128, 2 * T_PAD], FP8, tag="xtr", name=f"xtr_{blk}")
            Xtr_v = Xtr[:].rearrange("p (a c) -> p a c", a=2)
            nc.gpsimd.memset(Xtr_v[:, :, T_BLK:T_PAD], 0.0)
            for kb in range(2):
                nc.sync.dma_start(Xtr_v[:, kb, 0:T_BLK], xrn[:, kb, t0:t0 + T_BLK])
        for grp in range(NGRP):
            g0 = GW * grp
            # ---- Q.T/K.T staging for this group (halo of 32 for K reads) ----
            qks = [qk_pool.tile([128, GWH], BF16, tag=f"qk{ob}",
                                name=f"qk{ob}_{blk}_{grp}") for ob in range(4)]
            for c2 in range(2):
                cs = slice(g0 + GCH * c2, g0 + GCH * (c2 + 1))
                for ob in range(4):
                    qkp = ps_qk.tile([128, 512], F32, tag="qkp")
                    if fp8:
                        # x-residual omitted on the q/k path: the logits are
                        # tiny (sigma~0.1) so x-fp8 error is acceptable there
                        nc.tensor.matmul(qkp[:, 0:GCH], qkw_v[:, :, ob, :],
                                         Xt_v[:, :, cs], start=True, stop=True,
                                         perf_mode=DR)
                    else:
                        for kb in range(2):
                            nc.tensor.matmul(qkp[:, 0:GCH], qkw_v[:, kb, ob, :],
                                             Xt_v[:, kb, cs],
                                             start=(kb == 0), stop=(kb == 1))
                    dstq = qks[ob][:, GCH * c2:GCH * (c2 + 1)]
                    if ob < 2:
                        nc.vector.tensor_scalar(
                            dstq, qkp[:, 0:GCH], WSCI, qkvb_sb[:, ob:ob + 1],
                            ALU.mult, ALU.add)
                    else:
                        nc.scalar.activation(dstq, qkp[:, 0:GCH], AF.Identity,
                                             bias=qkvb_sb[:, ob:ob + 1],
                                             scale=WSCI)
            # ---- attention pairs: 4 duos, software-skewed one duo deep ----
            for d in range(4):
                pm2 = pm_pool.tile([128, 1664], BF16, tag="pm2",
                                   name=f"pm2_{blk}_{grp}_{d}")
                pm2_pad = pm2[0:98, 0:1664].rearrange(
                    "p (a c) -> p a c", a=2)[:, :, 784:832]
                nc.vector.memset(pm2_pad, 0.0)
                pex2 = pex_pool.tile([128, 1568], BF16, tag="pex2",
                                     name=f"pex2_{blk}_{grp}_{d}")
                pairs = []
                for p2 in range(2):
                    p8 = 2 * d + p2
                    u = GPAIR * grp + p8
                    cg = 98 * p8
                    tag = f"{blk}_{u}"
                    sps = ps_s.tile([128, 2048], F32, tag="sps",
                                    name="sps_" + tag)
                    # S.T: head h -> band r=h%4 -> bank r (concurrent band
                    # MMs must hit distinct banks)
                    for h in range(NH):
                        r = h % 4
                        b = h // 4
                        kt = qks[2 + b]
                        qt = qks[0 + b]
                        rs = slice(32 * r, 32 * r + 32)
                        nc.tensor.matmul(
                            sps[:, 512 * r + 98 * b:512 * r + 98 * b + 98],
                            kt[rs, cg:cg + 128], qt[rs, cg:cg + 98],
                            start=True, stop=True,
                            tile_position=(32 * r, 0))
                    # exp(S.T) on ACT, split per bank-pair so the next
                    # pair's band MMs can reuse banks 0-1 sooner
                    # pm slot s=2r+b holds head 4b+r
                    for hf in range(2):
                        sps_v = sps[0:98, 1024 * hf:1024 * hf + 1024].rearrange(
                            "p (a c) -> p a c", a=2)[:, :, 0:196]
                        pxv = pex2[0:98, 784 * p2 + 392 * hf:
                                   784 * p2 + 392 * (hf + 1)].rearrange(
                            "p (a c) -> p a c", a=2)
                        nc.scalar.activation(pxv, sps_v, AF.Exp)
                    # * exp(bias [+mask]) on the idle GpSimd engine
                    # (zeros kill the cross-window quadrants)
                    if mask_zero:
                        ebv = ebl_sb[0:98, :]
                    else:
                        m32 = (NPAIR * blk + u) % 32
                        ebv = ebl_sb[0:98, 784 * m32:784 * (m32 + 1)]
                    nc.gpsimd.tensor_mul(
                        pm2[0:98, 832 * p2:832 * p2 + 784],
                        pex2[0:98, 784 * p2:784 * p2 + 784], ebv)
                    # V in [token, channel-slot] layout
                    vw = ps_w.tile([128, 512], F32, tag="work",
                                   name="v_" + tag)
                    if fp8:
                        c0 = 98 * u
                        nc.tensor.matmul(vw[:, 0:272],
                                         Xt_v[:, :, c0:c0 + 128],
                                         wv_v, start=True, stop=False,
                                         perf_mode=DR)
                        nc.tensor.matmul(vw[:, 0:272],
                                         Xt_v[:, :, c0:c0 + 128],
                                         wvr_v, start=False, stop=False,
                                         perf_mode=DR)
                        nc.tensor.matmul(vw[:, 0:272],
                                         Xtr_v[:, :, c0:c0 + 128],
                                         wv_v, start=False, stop=True,
                                         perf_mode=DR)
                    else:
                        for kb in range(2):
                            nc.tensor.matmul(vw[:, 0:272],
                                             Xt_v[:, kb, 98 * u:98 * u + 128],
                                             wv_v[:, kb, :],
                                             start=(kb == 0), stop=(kb == 1))
                    vsb = v_pool.tile([128, 264], BF16, tag="vsb")
                    nc.vector.scalar_tensor_tensor(
                        vsb[0:98, :], vw[0:98, 0:264], WSCI, vbb_sb[0:98, :],
                        ALU.mult, ALU.add)
                    pairs.append(dict(vsb=vsb, tag=tag))
                st = hist.pop(0)
                if st is not None:
                    back_duo(st)
                    if st["d"] in (1, 3):
                        emit_proj(st)
                hist.append(dict(pm2=pm2, pairs=pairs, d=d,
                                 otsbG=otsbG if d > 0 else None,
                                 gt0=t0 + g0, tag=f"g{blk}_{grp}_{d}"))
                if d == 0:
                    otsbG = og_pool.tile([128, 2 * 784], BF16, tag="otsbG",
                                         name=f"og_{blk}_{grp}")
                    hist[-1]["otsbG"] = otsbG
    st = hist.pop(0)
    if st is not None:
        back_duo(st)
        if st["d"] in (1, 3):
            emit_proj(st)


_CACHED = {}


def _get_program(mask_zero: bool, fp8: bool):
    key = (mask_zero, fp8)
    if key in _CACHED:
        return _CACHED[key]
    XDT = FP8 if fp8 else BF16
    nc = bacc.Bacc("TRN2", target_bir_lowering=False, debug=False)
    ins = {
        "xin": nc.dram_tensor("xin", [128, 2, T_CORE], XDT,
                              kind="ExternalInput").ap(),
        "qkw": nc.dram_tensor("qkw", [128, 2, 4, 128], XDT,
                              kind="ExternalInput").ap(),
        "wv": nc.dram_tensor("wv", [128, 2, 272], XDT,
                             kind="ExternalInput").ap(),
        "pwt4": nc.dram_tensor("pwt4", [128, 2, 2, 128], BF16,
                               kind="ExternalInput").ap(),
        "identb": nc.dram_tensor("identb", [128, 128], BF16,
                                 kind="ExternalInput").ap(),
        "qkvb": nc.dram_tensor("qkvb", [128, 4], F32, kind="ExternalInput").ap(),
        "vbb": nc.dram_tensor("vbb", [128, 264], F32, kind="ExternalInput").ap(),
        "pbT": nc.dram_tensor("pbT", [128, 2], F32, kind="ExternalInput").ap(),
    }
    if fp8:
        ins["xrn"] = nc.dram_tensor("xrn", [128, 2, T_CORE], FP8,
                                    kind="ExternalInput").ap()
        ins["wvr"] = nc.dram_tensor("wvr", [128, 2, 272], FP8,
                                    kind="ExternalInput").ap()
    if mask_zero:
        ins["ebl"] = nc.dram_tensor("ebl", [128, 8, 98], BF16,
                                    kind="ExternalInput").ap()
    else:
        ins["ebl"] = nc.dram_tensor("ebl", [128, 32, 784], BF16,
                                    kind="ExternalInput").ap()
    outT = nc.dram_tensor("outT", [128, 2, T_CORE], BF16,
                          kind="ExternalOutput").ap()
    with tile.TileContext(nc) as tc:
        with ExitStack() as ctx:
            build_kernel(ctx, tc, ins, outT, mask_zero, fp8)
    nc.compile()
    _CACHED[key] = nc
    return nc


def _host_prep(mask, qkv_w, qkv_b, proj_w, proj_b, bias_table, fp8):
    xdt = ml_dtypes.float8_e4m3 if fp8 else ml_dtypes.bfloat16
    bf = ml_dtypes.bfloat16
    qkv_w = np.asarray(qkv_w, np.float32)
    qkv_b = np.asarray(qkv_b, np.float32)
    mask = np.asarray(mask, np.float32)
    mask_zero = not np.any(mask)

    wqk = qkv_w[0:512].copy()          # [512 oc, 256 ic]
    wqk[0:256] *= SCALE                # fold softmax scale into q
    # [ic, oc] -> [kb, 128ic, ob, 128oc] -> [128ic, kb, ob, 128oc]
    qkw = np.ascontiguousarray(
        (wqk.T * WSC).reshape(2, 128, 4, 128).transpose(1, 0, 2, 3)).astype(xdt)
    qb = qkv_b.copy()
    qb[0:256] *= SCALE
    qkvb = np.ascontiguousarray(qb[0:512].reshape(4, 128).T)   # [128, 4]

    wvT = qkv_w[512:768].T * WSC       # [256 ic, 256 oc]
    wv = np.zeros((2, 128, 272), np.float32)
    for h in range(NH):
        wv[:, :, 33 * h:33 * h + 32] = wvT.reshape(2, 128, 8, 32)[:, :, h]
    wv_f = np.ascontiguousarray(wv.transpose(1, 0, 2))
    wv = wv_f.astype(xdt)
    wvr = None
    if fp8:
        wvr = (wv_f - wv.astype(np.float32)).astype(ml_dtypes.float8_e4m3)
    vb = np.zeros((264,), np.float32)
    for h in range(NH):
        vb[33 * h:33 * h + 32] = qkv_b[512 + 32 * h:512 + 32 * h + 32]
        vb[33 * h + 32] = 1.0          # rowsum ones come from the bias add
    vbb = np.ascontiguousarray(np.broadcast_to(vb, (128, 264)))

    # proj W.T chunks for W-stationary proj: pwt4[p, i, o, m] = W[128o+m, 128i+p]
    pw = np.asarray(proj_w, np.float32)
    pwt4 = np.ascontiguousarray(
        pw.reshape(2, 128, 2, 128).transpose(3, 2, 0, 1)).astype(bf)
    pb = np.asarray(proj_b, np.float32)
    pbT = np.ascontiguousarray(pb.reshape(2, 128).T)  # [128, 2]

    rel = _rel_pos_index()
    bias_g = np.asarray(bias_table, np.float32)[rel].reshape(NTOK, NTOK, NH)  # [q,k,h]
    bT = bias_g.transpose(1, 2, 0)                    # [k, h, q]
    # pm slot s = 2r+b holds head 4b+r  ->  slot-order head permutation
    SLOT_HEADS = [0, 4, 1, 5, 2, 6, 3, 7]
    if mask_zero:
        ebl = np.zeros((128, NH, 98), np.float32)   # [k, slot, q-in-pair]
        ebl[0:49, :, 0:49] = np.exp(bT)
        ebl[49:98, :, 49:98] = np.exp(bT)
        ebl = np.ascontiguousarray(ebl[:, SLOT_HEADS, :]).astype(bf)
    else:
        ebl = np.zeros((128, 32, NH, 98), np.float32)
        for p in range(32):
            for w in range(2):
                cb = np.exp(bias_g + mask[2 * p + w][:, :, None]).transpose(1, 2, 0)
                ebl[49 * w:49 * w + 49, p, :, 49 * w:49 * w + 49] = cb
        ebl = np.ascontiguousarray(
            ebl[:, :, SLOT_HEADS, :].reshape(128, 32, 784)).astype(bf)
    identb = np.eye(128).astype(bf)
    return qkw, qkvb, wv, wvr, vbb, pwt4, pbT, ebl, identb, mask_zero


def kernel(x, mask, qkv_w, qkv_b, proj_w, proj_b, bias_table, _trace=False):
    fp8 = FP8_EN
    xdt = ml_dtypes.float8_e4m3 if fp8 else ml_dtypes.bfloat16
    (qkw, qkvb, wv, wvr, vbb, pwt4, pbT, ebl, identb,
     mask_zero) = _host_prep(mask, qkv_w, qkv_b, proj_w, proj_b, bias_table, fp8)
    # [T_full, 256] -> x.T [2kb, 128, cores, T_CORE] -> per-core [128, 2, T]
    xT = np.asarray(x, np.float32).reshape(B_FULL * NTOK, DIM).T
    xTf = xT.reshape(2, 128, NCORES, T_CORE).transpose(1, 0, 2, 3)
    xTs = xTf.astype(xdt)
    xTr = None
    if fp8:
        xTr = (xTf - xTs.astype(np.float32)).astype(ml_dtypes.float8_e4m3)
    in_maps = []
    for c in range(NCORES):
        m = {"xin": np.ascontiguousarray(xTs[:, :, c, :]), "qkw": qkw,
             "wv": wv, "pwt4": pwt4, "ebl": ebl,
             "identb": identb, "qkvb": qkvb, "vbb": vbb, "pbT": pbT}
        if fp8:
            m["xrn"] = np.ascontiguousarray(xTr[:, :, c, :])
            m["wvr"] = wvr
        in_maps.append(m)
    nc = _get_program(mask_zero, fp8)
    res = bass_utils.run_bass_kernel_spmd(nc, in_maps, core_ids=list(range(NCORES)),
                                          trace=_trace)
    outT = np.stack([np.asarray(r["outT"]) for r in res.results])  # [8, 128, 2, T]
    # channel oc = 128*o + p  ->  [8, 256, T] -> [8, T, 256]
    out = outT.transpose(0, 2, 1, 3).reshape(NCORES, 256, T_CORE)
    out = out.transpose(0, 2, 1).astype(np.float32)
    out = np.ascontiguousarray(out.reshape(B_FULL, NTOK, DIM))
    if _trace:
        kernel.last_results = res
    return out


# revision 48
# speedup vs baseline: 1.0542x; 1.0542x over previous
"""Trainium2 Bass kernel for Swin-style window attention (MegatronWindowAttention).

Per window w (49 tokens, dim 256, 8 heads x 32):
  qkv = x @ qkv_w.T + qkv_b ; attn = softmax(q*scale @ k.T + bias + mask[w%64])
  out = (attn @ v) @ proj_w.T + proj_b

Sharding: data-parallel over B_=4096 windows across 8 cores (512 windows each).

v2 device dataflow per core (4 blocks of 128 windows; window PAIRS of 98 tokens):
  - x.T arrives fp8 (e4m3) kb-interleaved [128, 2, T]; host pre-transposed
  - Q.T/K.T staged per 8-pair group via fp8 DoubleRow W-stationary matmuls
    (both 128-ic halves in one MM); PSUM evac: q on DVE (scale+bias), k on ACT
  - Per pair: log-domain rel-pos bias (+mask, -30 cross-window kill) PRELOADED
    into the S.T PSUM banks via identity matmuls; S.T matmuls accumulate on
    top (4 row-band-concurrent per bank); ONE ACT exp -> pm (attn weights)
  - V per pair via ONE fp8 DoubleRow MM (slotted 33-wide per head; ones col
    comes free from the bias add); evac fused scale+bias on DVE
  - AV per head: lhsT = pm 128-col slot (FWL), rhs = V_aug -> O + rowsum
  - normalize via DVE reciprocal+mul; PE transposes (2/pair) -> O.T packed
    per-group; proj is W-stationary per half-group (4 MMs of 392 cols)
    producing OUT.T; ACT evac (+proj bias per-partition); bf16 DMA out.
  - host reassembles out.T -> [B, N, 256] f32
"""

import os
import numpy as np
import ml_dtypes
from contextlib import ExitStack

import concourse.bass as bass
import concourse.tile as tile
import concourse.mybir as mybir
from concourse import bacc
from concourse import bass_utils

FP8_EN = os.environ.get("FP8", "1") == "1"

WH = WW = 7
NTOK = 49
DIM = 256
NH = 8
HD = 32
SCALE = HD ** -0.5
NCORES = 8
B_FULL = 4096
NW = 64
B_CORE = B_FULL // NCORES          # 512 windows per core
T_CORE = B_CORE * NTOK             # 25088 tokens
NBLOCK = 4
W_BLK = 128                        # windows per block
T_BLK = W_BLK * NTOK               # 6272 tokens per block
T_PAD = T_BLK + 32                 # tail pad for 128-col stationary reads
NPAIR = W_BLK // 2                 # 64 pairs per block
GPAIR = 8                          # pairs per QK staging group
NGRP = NPAIR // GPAIR              # 8 groups per block
GW = GPAIR * 98                    # 784 tokens per group
GWH = GW + 32                      # halo for 128-col K stationary reads
GCH = GWH // 2                     # 408: QK psum chunk width
WSC = 32.0                         # fp8 weight pre-scale (folded out in evac)
WSCI = 1.0 / WSC

F32 = mybir.dt.float32
BF16 = mybir.dt.bfloat16
FP8 = mybir.dt.float8e4
AF = mybir.ActivationFunctionType
ALU = mybir.AluOpType
DR = mybir.MatmulPerfMode.DoubleRow


def _rel_pos_index():
    coords = np.stack(np.meshgrid(np.arange(WH), np.arange(WW), indexing='ij'))
    flat = coords.reshape(2, -1)
    rel = flat[:, :, None] - flat[:, None, :]
    rel = rel.transpose(1, 2, 0).copy()
    rel[:, :, 0] += WH - 1
    rel[:, :, 1] += WW - 1
    rel[:, :, 0] *= 2 * WW - 1
    return rel.sum(-1).reshape(-1)


def build_kernel(ctx: ExitStack, tc: tile.TileContext, ins: dict, outT: bass.AP,
                 mask_zero: bool, fp8: bool):
    nc = tc.nc
    XDT = FP8 if fp8 else BF16
    xin = ins["xin"]        # [128, 2, T_CORE] fp8/bf16 (x.T, kb-interleaved)
    xrn = ins.get("xrn")    # [128, 2, T_CORE] fp8 residual x.T (fp8 path only)
    qkw = ins["qkw"]        # [128, 2, 4, 128] fp8/bf16 (ic, kb, ob(q0 q1 k0 k1), oc)
    wv = ins["wv"]          # [128, 2, 272] fp8/bf16 (ic, kb, slotted oc)
    wvr = ins.get("wvr")    # [128, 2, 272] fp8 residual V weights (fp8 path only)
    pwt4 = ins["pwt4"]      # [128, 2, 2, 128] bf16 proj W.T chunks (ic-chunk, oc-chunk)
    ebl = ins["ebl"]        # log-domain bias [128, 2, 392] (fast) or [128, 32, 2, 392]
    idb = ins["identb"]     # [128, 128] bf16
    qkvb = ins["qkvb"]      # [128, 4] f32 per-partition bias for q0 q1 k0 k1
    vbb = ins["vbb"]        # [128, 264] f32 broadcast v-bias (+1.0 ones slots)
    pbT = ins["pbT"]        # [128, 2] f32 proj bias transposed chunks

    const = ctx.enter_context(tc.tile_pool(name="const", bufs=1))
    qkw_sb = const.tile([128, 2 * 4 * 128], XDT, tag="qkw")
    nc.sync.dma_start(qkw_sb[:], qkw.rearrange("p a b c -> p (a b c)"))
    qkw_v = qkw_sb[:].rearrange("p (a b c) -> p a b c", a=2, b=4)
    wv_sb = const.tile([128, 2 * 272], XDT, tag="wv")
    nc.sync.dma_start(wv_sb[:], wv.rearrange("p a c -> p (a c)"))
    wv_v = wv_sb[:].rearrange("p (a c) -> p a c", a=2)
    if fp8:
        wvr_sb = const.tile([128, 2 * 272], FP8, tag="wvr")
        nc.sync.dma_start(wvr_sb[:], wvr.rearrange("p a c -> p (a c)"))
        wvr_v = wvr_sb[:].rearrange("p (a c) -> p a c", a=2)
    pwt_sb = const.tile([128, 4 * 128], BF16, tag="pwt4")
    nc.sync.dma_start(pwt_sb[:], pwt4.rearrange("p a b c -> p (a b c)"))
    idb_sb = const.tile([128, 128], BF16, tag="identb")
    nc.sync.dma_start(idb_sb[:], idb[:])
    qkvb_sb = const.tile([128, 4], F32, tag="qkvb")
    nc.sync.dma_start(qkvb_sb[:], qkvb[:])
    vbb_sb = const.tile([128, 264], F32, tag="vbb")
    nc.sync.dma_start(vbb_sb[:], vbb[:])
    pbT_sb = const.tile([128, 2], F32, tag="pbT")
    nc.sync.dma_start(pbT_sb[:], pbT[:])
    if mask_zero:
        ebl_sb = const.tile([128, 784], BF16, tag="ebm")
        nc.sync.dma_start(ebl_sb[:], ebl.rearrange("p a c -> p (a c)"))
    else:
        ebl_sb = const.tile([128, 32 * 784], BF16, tag="ebm32")
        nc.sync.dma_start(ebl_sb[:], ebl.rearrange("p m c -> p (m c)"))

    xt_pool = ctx.enter_context(tc.tile_pool(name="xt", bufs=2))
    qk_pool = ctx.enter_context(tc.tile_pool(name="qk", bufs=2))
    pm_pool = ctx.enter_context(tc.tile_pool(name="pm", bufs=2))
    pex_pool = ctx.enter_context(tc.tile_pool(name="pex", bufs=2))
    v_pool = ctx.enter_context(tc.tile_pool(name="vsb", bufs=4))
    r_pool = ctx.enter_context(tc.tile_pool(name="rsb", bufs=4))
    on_pool = ctx.enter_context(tc.tile_pool(name="onorm", bufs=4))
    og_pool = ctx.enter_context(tc.tile_pool(name="otsbG", bufs=2))
    osb_pool = ctx.enter_context(tc.tile_pool(name="osbG", bufs=2))

    # 8 PSUM banks: sps 1x4 (S.T + preloaded bias; concurrent row-band MMs
    # must hit DISTINCT banks -> bank r holds heads (r, r+4)), qkp 2x1
    # (staging + proj), work 2x1 (vw -> avw -> otw rotation)
    ps_s = ctx.enter_context(tc.tile_pool(name="pss", bufs=1, space="PSUM"))
    ps_qk = ctx.enter_context(tc.tile_pool(name="psqk", bufs=2, space="PSUM"))
    ps_w = ctx.enter_context(tc.tile_pool(name="psw", bufs=2, space="PSUM"))

    hist = [None]   # duo-level software pipeline (back half one duo late)

    def back_duo(st):
        pm2 = st["pm2"]
        for p2, sub in enumerate(st["pairs"]):
            vsb = sub["vsb"]
            avw = ps_w.tile([128, 512], F32, tag="work", name="av_" + sub["tag"])
            for h in range(NH):
                s_h = 2 * (h % 4) + h // 4   # pm slot holding head h
                nc.tensor.matmul(
                    avw[:, 33 * h:33 * h + 33],
                    pm2[0:98, 832 * p2 + 98 * s_h:832 * p2 + 98 * s_h + 128],
                    vsb[0:98, 33 * h:33 * h + 33], start=True, stop=True)
            av_v = avw[:, 0:264].rearrange("p (h c) -> p h c", h=8)
            recip = r_pool.tile([128, 8], F32, tag="recip")
            nc.vector.reciprocal(recip[0:98, :], av_v[0:98, :, 32])
            onorm = on_pool.tile([128, 256], BF16, tag="onorm")
            onorm_v = onorm[:].rearrange("p (h c) -> p h c", h=8)
            recip_b = recip[0:98, :].unsqueeze(2).broadcast_to([98, 8, 32])
            nc.vector.tensor_mul(onorm_v[0:98, :, :], av_v[0:98, :, 0:32], recip_b)
            sub["onorm"] = onorm
        # O.T for both pairs into one bank: col = 196*p2 + 98*hb
        otw = ps_w.tile([128, 512], F32, tag="work", name="ot_" + st["tag"])
        otp = otw[:].bitcast(BF16)
        for p2, sub in enumerate(st["pairs"]):
            for hb in range(2):
                nc.tensor.transpose(
                    otp[:, 196 * p2 + 98 * hb:196 * p2 + 98 * hb + 98],
                    sub["onorm"][0:98, 128 * hb:128 * (hb + 1)],
                    idb_sb[0:98, 0:98])
        # evacuate to the group O.T buffer (ch-half major), DVE 2x bf16
        in_v = otp[:, 0:392].rearrange("p (b a c) -> p a b c", b=2, a=2)
        og_v = st["otsbG"][:].rearrange("p (a c) -> p a c", a=2)
        d = st["d"]
        out_v = og_v[:, :, 196 * d:196 * d + 196].rearrange(
            "p a (b c) -> p a b c", b=2)
        nc.vector.tensor_copy(out_v, in_v)

    def emit_proj(st):
        # W-stationary proj for one half-group (392 tokens) -> OUT.T, evac, DMA
        hg = st["d"] // 2
        og_v = st["otsbG"][:].rearrange("p (a c) -> p a c", a=2)
        osb = osb_pool.tile([128, 2 * 392], BF16, tag="osbG")
        osb_v = osb[:].rearrange("p (a c) -> p a c", a=2)
        for o in range(2):
            pj = ps_qk.tile([128, 512], F32, tag="qkp",
                            name=f"pj{o}_" + st["tag"])
            for i in range(2):
                nc.tensor.matmul(
                    pj[:, 0:392], pwt_sb[:, 128 * (2 * i + o):128 * (2 * i + o + 1)],
                    og_v[:, i, 392 * hg:392 * (hg + 1)],
                    start=(i == 0), stop=(i == 1))
            nc.scalar.activation(osb_v[:, o, :], pj[:, 0:392], AF.Identity,
                                 bias=pbT_sb[:, o:o + 1])
        gt0 = st["gt0"] + 392 * hg
        for o in range(2):
            nc.sync.dma_start(outT[:, o, gt0:gt0 + 392], osb_v[:, o, :])

    for blk in range(NBLOCK):
        t0 = blk * T_BLK
        Xt = xt_pool.tile([128, 2 * T_PAD], XDT, tag="xt", name=f"xt_{blk}")
        Xt_v = Xt[:].rearrange("p (a c) -> p a c", a=2)
        nc.gpsimd.memset(Xt_v[:, :, T_BLK:T_PAD], 0.0)
        for kb in range(2):
            nc.sync.dma_start(Xt_v[:, kb, 0:T_BLK], xin[:, kb, t0:t0 + T_BLK])
        if fp8:
            Xtr = xt_pool.tile([128, 2 * T_PAD], FP8, tag="xtr", name=f"xtr_{blk}")
            Xtr_v = Xtr[:].rearrange("p (a c) -> p a c", a=2)
            nc.gpsimd.memset(Xtr_v[:, :, T_BLK:T_PAD], 0.0)
            for kb in range(2):
                nc.sync.dma_start(Xtr_v[:, kb, 0:T_BLK], xrn[:, kb, t0:t0 + T_BLK])
        for grp in range(NGRP):
            g0 = GW * grp
            # ---- Q.T/K.T staging for this group (halo of 32 for K reads) ----
            qks = [qk_pool.tile([128, GWH], BF16, tag=f"qk{ob}",
                                name=f"qk{ob}_{blk}_{grp}") for ob in range(4)]
            for c2 in range(2):
                cs = slice(g0 + GCH * c2, g0 + GCH * (c2 + 1))
                for ob in range(4):
                    qkp = ps_qk.tile([128, 512], F32, tag="qkp")
                    if fp8:
                        # x-residual omitted on the q/k path: the logits are
                        # tiny (sigma~0.1) so x-fp8 error is acceptable there
                        nc.tensor.matmul(qkp[:, 0:GCH], qkw_v[:, :, ob, :],
                                         Xt_v[:, :, cs], start=True, stop=True,
                                         perf_mode=DR)
                    else:
                        for kb in range(2):
                            nc.tensor.matmul(qkp[:, 0:GCH], qkw_v[:, kb, ob, :],
                                             Xt_v[:, kb, cs],
                                             start=(kb == 0), stop=(kb == 1))
                    dstq = qks[ob][:, GCH * c2:GCH * (c2 + 1)]
                    if ob < 2:
                        nc.vector.tensor_scalar(
                            dstq, qkp[:, 0:GCH], WSCI, qkvb_sb[:, ob:ob + 1],
                            ALU.mult, ALU.add)
                    else:
                        nc.scalar.activation(dstq, qkp[:, 0:GCH], AF.Identity,
                                             bias=qkvb_sb[:, ob:ob + 1],
                                             scale=WSCI)
            # ---- attention pairs: 4 duos, software-skewed one duo deep ----
            for d in range(4):
                pm2 = pm_pool.tile([128, 1664], BF16, tag="pm2",
                                   name=f"pm2_{blk}_{grp}_{d}")
                pm2_pad = pm2[0:98, 0:1664].rearrange(
                    "p (a c) -> p a c", a=2)[:, :, 784:832]
                nc.vector.memset(pm2_pad, 0.0)
                pex2 = pex_pool.tile([128, 1568], BF16, tag="pex2",
                                     name=f"pex2_{blk}_{grp}_{d}")
                pairs = []
                for p2 in range(2):
                    p8 = 2 * d + p2
                    u = GPAIR * grp + p8
                    cg = 98 * p8
                    tag = f"{blk}_{u}"
                    sps = ps_s.tile([128, 2048], F32, tag="sps",
                                    name="sps_" + tag)
                    # S.T: head h -> band r=h%4 -> bank r (concurrent band
                    # MMs must hit distinct banks)
                    for h in range(NH):
                        r = h % 4
                        b = h // 4
                        kt = qks[2 + b]
                        qt = qks[0 + b]
                        rs = slice(32 * r, 32 * r + 32)
                        nc.tensor.matmul(
                            sps[:, 512 * r + 98 * b:512 * r + 98 * b + 98],
                            kt[rs, cg:cg + 128], qt[rs, cg:cg + 98],
                            start=True, stop=True,
                            tile_position=(32 * r, 0))
                    # exp(S.T) on ACT; pm slot s=2r+b holds head 4b+r
                    sps_v = sps[0:98, :].rearrange("p (a c) -> p a c",
                                                   a=4)[:, :, 0:196]
                    pxv = pex2[0:98, 784 * p2:784 * p2 + 784].rearrange(
                        "p (a c) -> p a c", a=4)
                    nc.scalar.activation(pxv, sps_v, AF.Exp)
                    # * exp(bias [+mask]) on the idle GpSimd engine
                    # (zeros kill the cross-window quadrants)
                    if mask_zero:
                        ebv = ebl_sb[0:98, :]
                    else:
                        m32 = (NPAIR * blk + u) % 32
                        ebv = ebl_sb[0:98, 784 * m32:784 * (m32 + 1)]
                    nc.gpsimd.tensor_mul(
                        pm2[0:98, 832 * p2:832 * p2 + 784],
                        pex2[0:98, 784 * p2:784 * p2 + 784], ebv)
                    # V in [token, channel-slot] layout
                    vw = ps_w.tile([128, 512], F32, tag="work",
                                   name="v_" + tag)
                    if fp8:
                        c0 = 98 * u
                        nc.tensor.matmul(vw[:, 0:272],
                                         Xt_v[:, :, c0:c0 + 128],
                                         wv_v, start=True, stop=False,
                                         perf_mode=DR)
                        nc.tensor.matmul(vw[:, 0:272],
                                         Xt_v[:, :, c0:c0 + 128],
                                         wvr_v, start=False, stop=False,
                                         perf_mode=DR)
                        nc.tensor.matmul(vw[:, 0:272],
                                         Xtr_v[:, :, c0:c0 + 128],
                                         wv_v, start=False, stop=True,
                                         perf_mode=DR)
                    else:
                        for kb in range(2):
                            nc.tensor.matmul(vw[:, 0:272],
                                             Xt_v[:, kb, 98 * u:98 * u + 128],
                                             wv_v[:, kb, :],
                                             start=(kb == 0), stop=(kb == 1))
                    vsb = v_pool.tile([128, 264], BF16, tag="vsb")
                    nc.vector.scalar_tensor_tensor(
                        vsb[0:98, :], vw[0:98, 0:264], WSCI, vbb_sb[0:98, :],
                        ALU.mult, ALU.add)
                    pairs.append(dict(vsb=vsb, tag=tag))
                st = hist.pop(0)
                if st is not None:
                    back_duo(st)
                    if st["d"] in (1, 3):
                        emit_proj(st)
                hist.append(dict(pm2=pm2, pairs=pairs, d=d,
                                 otsbG=otsbG if d > 0 else None,
                                 gt0=t0 + g0, tag=f"g{blk}_{grp}_{d}"))
                if d == 0:
                    otsbG = og_pool.tile([128, 2 * 784], BF16, tag="otsbG",
                                         name=f"og_{blk}_{grp}")
                    hist[-1]["otsbG"] = otsbG
    st = hist.pop(0)
    if st is not None:
        back_duo(st)
        if st["d"] in (1, 3):
            emit_proj(st)


_CACHED = {}


def _get_program(mask_zero: bool, fp8: bool):
    key = (mask_zero, fp8)
    if key in _CACHED:
        return _CACHED[key]
    XDT = FP8 if fp8 else BF16
    nc = bacc.Bacc("TRN2", target_bir_lowering=False, debug=False)
    ins = {
        "xin": nc.dram_tensor("xin", [128, 2, T_CORE], XDT,
                              kind="ExternalInput").ap(),
        "qkw": nc.dram_tensor("qkw", [128, 2, 4, 128], XDT,
                              kind="ExternalInput").ap(),
        "wv": nc.dram_tensor("wv", [128, 2, 272], XDT,
                             kind="ExternalInput").ap(),
        "pwt4": nc.dram_tensor("pwt4", [128, 2, 2, 128], BF16,
                               kind="ExternalInput").ap(),
        "identb": nc.dram_tensor("identb", [128, 128], BF16,
                                 kind="ExternalInput").ap(),
        "qkvb": nc.dram_tensor("qkvb", [128, 4], F32, kind="ExternalInput").ap(),
        "vbb": nc.dram_tensor("vbb", [128, 264], F32, kind="ExternalInput").ap(),
        "pbT": nc.dram_tensor("pbT", [128, 2], F32, kind="ExternalInput").ap(),
    }
    if fp8:
        ins["xrn"] = nc.dram_tensor("xrn", [128, 2, T_CORE], FP8,
                                    kind="ExternalInput").ap()
        ins["wvr"] = nc.dram_tensor("wvr", [128, 2, 272], FP8,
                                    kind="ExternalInput").ap()
    if mask_zero:
        ins["ebl"] = nc.dram_tensor("ebl", [128, 8, 98], BF16,
                                    kind="ExternalInput").ap()
    else:
        ins["ebl"] = nc.dram_tensor("ebl", [128, 32, 784], BF16,
                                    kind="ExternalInput").ap()
    outT = nc.dram_tensor("outT", [128, 2, T_CORE], BF16,
                          kind="ExternalOutput").ap()
    with tile.TileContext(nc) as tc:
        with ExitStack() as ctx:
            build_kernel(ctx, tc, ins, outT, mask_zero, fp8)
    nc.compile()
    _CACHED[key] = nc
    return nc


def _host_prep(mask, qkv_w, qkv_b, proj_w, proj_b, bias_table, fp8):
    xdt = ml_dtypes.float8_e4m3 if fp8 else ml_dtypes.bfloat16
    bf = ml_dtypes.bfloat16
    qkv_w = np.asarray(qkv_w, np.float32)
    qkv_b = np.asarray(qkv_b, np.float32)
    mask = np.asarray(mask, np.float32)
    mask_zero = not np.any(mask)

    wqk = qkv_w[0:512].copy()          # [512 oc, 256 ic]
    wqk[0:256] *= SCALE                # fold softmax scale into q
    # [ic, oc] -> [kb, 128ic, ob, 128oc] -> [128ic, kb, ob, 128oc]
    qkw = np.ascontiguousarray(
        (wqk.T * WSC).reshape(2, 128, 4, 128).transpose(1, 0, 2, 3)).astype(xdt)
    qb = qkv_b.copy()
    qb[0:256] *= SCALE
    qkvb = np.ascontiguousarray(qb[0:512].reshape(4, 128).T)   # [128, 4]

    wvT = qkv_w[512:768].T * WSC       # [256 ic, 256 oc]
    wv = np.zeros((2, 128, 272), np.float32)
    for h in range(NH):
        wv[:, :, 33 * h:33 * h + 32] = wvT.reshape(2, 128, 8, 32)[:, :, h]
    wv_f = np.ascontiguousarray(wv.transpose(1, 0, 2))
    wv = wv_f.astype(xdt)
    wvr = None
    if fp8:
        wvr = (wv_f - wv.astype(np.float32)).astype(ml_dtypes.float8_e4m3)
    vb = np.zeros((264,), np.float32)
    for h in range(NH):
        vb[33 * h:33 * h + 32] = qkv_b[512 + 32 * h:512 + 32 * h + 32]
        vb[33 * h + 32] = 1.0          # rowsum ones come from the bias add
    vbb = np.ascontiguousarray(np.broadcast_to(vb, (128, 264)))

    # proj W.T chunks for W-stationary proj: pwt4[p, i, o, m] = W[128o+m, 128i+p]
    pw = np.asarray(proj_w, np.float32)
    pwt4 = np.ascontiguousarray(
        pw.reshape(2, 128, 2, 128).transpose(3, 2, 0, 1)).astype(bf)
    pb = np.asarray(proj_b, np.float32)
    pbT = np.ascontiguousarray(pb.reshape(2, 128).T)  # [128, 2]

    rel = _rel_pos_index()
    bias_g = np.asarray(bias_table, np.float32)[rel].reshape(NTOK, NTOK, NH)  # [q,k,h]
    bT = bias_g.transpose(1, 2, 0)                    # [k, h, q]
    # pm slot s = 2r+b holds head 4b+r  ->  slot-order head permutation
    SLOT_HEADS = [0, 4, 1, 5, 2, 6, 3, 7]
    if mask_zero:
        ebl = np.zeros((128, NH, 98), np.float32)   # [k, slot, q-in-pair]
        ebl[0:49, :, 0:49] = np.exp(bT)
        ebl[49:98, :, 49:98] = np.exp(bT)
        ebl = np.ascontiguousarray(ebl[:, SLOT_HEADS, :]).astype(bf)
    else:
        ebl = np.zeros((128, 32, NH, 98), np.float32)
        for p in range(32):
            for w in range(2):
                cb = np.exp(bias_g + mask[2 * p + w][:, :, None]).transpose(1, 2, 0)
                ebl[49 * w:49 * w + 49, p, :, 49 * w:49 * w + 49] = cb
        ebl = np.ascontiguousarray(
            ebl[:, :, SLOT_HEADS, :].reshape(128, 32, 784)).astype(bf)
    identb = np.eye(128).astype(bf)
    return qkw, qkvb, wv, wvr, vbb, pwt4, pbT, ebl, identb, mask_zero


def kernel(x, mask, qkv_w, qkv_b, proj_w, proj_b, bias_table, _trace=False):
    fp8 = FP8_EN
    xdt = ml_dtypes.float8_e4m3 if fp8 else ml_dtypes.bfloat16
    (qkw, qkvb, wv, wvr, vbb, pwt4, pbT, ebl, identb,
     mask_zero) = _host_prep(mask, qkv_w, qkv_b, proj_w, proj_b, bias_table, fp8)
    # [T_full, 256] -> x.T [2kb, 128, cores, T_CORE] -> per-core [128, 2, T]
    xT = np.asarray(x, np.float32).reshape(B_FULL * NTOK, DIM).T
    xTf = xT.reshape(2, 128, NCORES, T_CORE).transpose(1, 0, 2, 3)
    xTs = xTf.astype(xdt)
    xTr = None
    if fp8:
        xTr = (xTf - xTs.astype(np.float32)).astype(ml_dtypes.float8_e4m3)
    in_maps = []
    for c in range(NCORES):
        m = {"xin": np.ascontiguousarray(xTs[:, :, c, :]), "qkw": qkw,
             "wv": wv, "pwt4": pwt4, "ebl": ebl,
             "identb": identb, "qkvb": qkvb, "vbb": vbb, "pbT": pbT}
        if fp8:
            m["xrn"] = np.ascontiguousarray(xTr[:, :, c, :])
            m["wvr"] = wvr
        in_maps.append(m)
    nc = _get_program(mask_zero, fp8)
    res = bass_utils.run_bass_kernel_spmd(nc, in_maps, core_ids=list(range(NCORES)),
                                          trace=_trace)
    outT = np.stack([np.asarray(r["outT"]) for r in res.results])  # [8, 128, 2, T]
    # channel oc = 128*o + p  ->  [8, 256, T] -> [8, T, 256]
    out = outT.transpose(0, 2, 1, 3).reshape(NCORES, 256, T_CORE)
    out = out.transpose(0, 2, 1).astype(np.float32)
    out = np.ascontiguousarray(out.reshape(B_FULL, NTOK, DIM))
    if _trace:
        kernel.last_results = res
    return out
